# revision 1
# baseline (speedup 1.0000x reference)
"""Trainium2 Bass kernel for the ExponentialEnvelopes module.

Math (per spin):
    feats[n,k]  = [charge, centered coords]           (nuclei features, [128, 4])
    Z[n,o]      = (feats @ W_pi)[n,o]                 (= zeta.T)
    P[n,o]      = (feats @ W_zeta)[n,o]               (= pi.T)
    d[e,n]      = ||e_coords[e] - nuc_coords[n]||
    orb[e,o]    = sum_n P[n,o] * exp(-d[e,n] * |Z[n,o]|)
    out[s,det,e,me] = orb reshaped

All masks are all-ones for this problem (spec fill="ones"), so the masked
branches of the reference collapse to the above.

Sharding: electrons are sharded across the 8 cores (16 electrons/core, both
spins), orbitals (4096) are kept whole per core.  This gives each exp
activation instruction a 4096-wide free dim with the per-electron distance
applied through the ACT engine's per-partition `scale` operand, so the
[nuclei x orbital] outer product inside the exponent costs zero extra
instructions.  Per electron (steady state, ACT-bound at ~3.7us/electron):
    ACT : T = exp(absZ * (-d[:,e]))       [128, 4096] fp16   (~3.7us)
    DVE : T *= piT (in place, quarters)   [128, 4096] fp16   (~2.8us)
    PE  : 8x matmul(lhsT=onehot[128,16], rhs=T chunk [128,512])
          -> accumulates the partition-reduction into PSUM row e of the
          per-chunk [16, 512] accumulator (start at e=0, stop at e=15)
Host gathers the per-core [2, 16, 4096] slabs (already in orb layout).

Measured: ~156us HW exec on 8 cores (scalar-engine exp floor is ~119us;
NEFF preamble + zeta/pi setup head ~15us, drain/barrier tail ~20us).
"""

import numpy as np
from contextlib import ExitStack

NE = 128          # electrons per spin (total)
NN = 128          # nuclei
NDET = 32
NORB = 4096       # n_det * max_e
N_CORES = 8
E_PER_CORE = NE // N_CORES   # 16
NBLK = NORB // 128
WBLK = NORB // 512           # zeta/pi matmul blocks of 512

_CACHE = {}

LAST_RESULTS = None  # BassKernelResults of the most recent run (for test harness)


def _split_multiwaits(nc, blocks):
    """Every TPB engine instruction has exactly ONE embedded sync-wait slot
    (NEURON_ISA_TPB_EVENTS); Tile's sem assignment can emit several waits on
    one instruction, which walrus rejects ("Too many sync wait commands").
    Hoist all but the last wait onto fresh single-wait NOPs inserted just
    before the instruction on the same engine stream."""
    from concourse import mybir

    for bb, insts in blocks.items():
        out = []
        changed = False
        for inst in insts:
            si = getattr(inst, "sync_info", None)
            waits = list(si.on_wait) if si is not None and si.on_wait else []
            if len(waits) > 1:
                for w in waits[:-1]:
                    nop = mybir.InstNoOp(
                        name=nc.get_next_instruction_name(), ins=[], outs=[])
                    nop.engine = inst.engine
                    nop.sync_info = mybir.SyncInfo(on_wait=[w], on_update=[])
                    out.append(nop)
                inst.sync_info = mybir.SyncInfo(
                    on_wait=[waits[-1]], on_update=list(si.on_update))
                changed = True
            out.append(inst)
        if changed:
            insts[:] = out


def _build_module():
    import concourse.bass as bass
    import concourse.tile as tile
    from concourse import mybir
    from concourse.alu_op_type import AluOpType

    class FixupTileContext(tile.TileContext):
        def _lower_ordered_insts(self, postordered_blocks):
            _split_multiwaits(self.nc, postordered_blocks)
            return super()._lower_ordered_insts(postordered_blocks)

        def _drain_and_barrier(self, tick_clock, wait_clock):
            # The kernel-tail drain waits on the full global clock (~11 sems),
            # over the single embedded wait slot.  Pre-observe the clock on
            # the sync engine via single-wait NOPs; add_sem_waits then elides
            # the (now redundant) waits on the real drain.
            from concourse.vector_clock import ScopedClock

            probe = self.nc.sync.nop()
            wait_clock.add_sem_waits(
                probe.ins, ScopedClock({None: tick_clock.global_clock}))
            si = probe.ins.sync_info
            waits = list(si.on_wait) if si is not None and si.on_wait else []
            if len(waits) > 1:
                probe.ins.sync_info = mybir.SyncInfo(
                    on_wait=[waits[0]], on_update=list(si.on_update or []))
                for w in waits[1:]:
                    extra = self.nc.sync.nop()
                    extra.ins.sync_info = mybir.SyncInfo(
                        on_wait=[w], on_update=[])
            ret = super()._drain_and_barrier(tick_clock, wait_clock)
            # The probes above pre-observed the whole clock on SP in program
            # order, so the tail drain's own waits are redundant — and exceed
            # the single embedded wait slot.  Strip them.
            for blk in self.nc.m.functions[0].blocks:
                for i in blk.instructions:
                    si = getattr(i, "sync_info", None)
                    if (isinstance(i, mybir.InstDrain) and si is not None
                            and si.on_wait and len(si.on_wait) > 1):
                        i.sync_info = mybir.SyncInfo(
                            on_wait=[], on_update=list(si.on_update or []))
            return ret

    f32 = mybir.dt.float32
    f16 = mybir.dt.float16
    AF = mybir.ActivationFunctionType
    AX = mybir.AxisListType.X
    E = E_PER_CORE

    nc = bass.Bass(trn_type="TRN2")

    # all small inputs packed into one DMA: [3, 288] =
    #   [:, 0:128] nucT rows, [0, 128:256] charges, [:, 256:272] eT_up,
    #   [:, 272:288] eT_dn  (all slices start at partition 0)
    d_small = nc.dram_tensor("small", [3, 2 * NN + 2 * E], f32,
                             kind="ExternalInput")
    # W matrices pre-split by the host into charge rows (k=0) and coord rows
    # (k=1..3) so every SBUF access pattern starts at partition 0; all four
    # matrices are packed along the free dim: index (s, m) at (2*s+m)*NORB.
    d_w4 = nc.dram_tensor("w4", [4, 4 * NORB], f16, kind="ExternalInput")
    # per-core output slab: [spin][e_local][orbital] (directly in orb layout)
    d_out = nc.dram_tensor("out", [2, E, NORB], f32, kind="ExternalOutput")

    with ExitStack() as ctx:
        tc = ctx.enter_context(FixupTileContext(nc))
        const = ctx.enter_context(tc.tile_pool(name="const", bufs=1))
        wpool = ctx.enter_context(tc.tile_pool(name="wload", bufs=1))
        tpool = ctx.enter_context(tc.tile_pool(name="texp", bufs=5))
        opool = ctx.enter_context(tc.tile_pool(name="outsb", bufs=8))
        psum = ctx.enter_context(tc.tile_pool(name="ps", bufs=1, space="PSUM"))
        # round-robin bank tags for transient setup psum tiles
        _bk = [0]

        def ps_tile(shape, tag=None):
            if tag is None:
                tag = f"bk{_bk[0] % 8}"
            _bk[0] += 1
            return psum.tile(shape, f32, tag=tag, name=f"ps{_bk[0]}_{tag}")

        # ---------------- small loads (single DMA) ----------------
        s_small = const.tile([3, 2 * NN + 2 * E], f32, tag="small")
        nc.sync.dma_start(s_small[:], d_small[:])
        s_nucT = s_small[:, 0:NN]
        s_chg = s_small[0:1, NN:2 * NN]
        s_eT = [s_small[:, 2 * NN:2 * NN + E],
                s_small[:, 2 * NN + E:2 * NN + 2 * E]]
        s_cnuc = const.tile([3, NN], f32, tag="cnuc")  # centered coords
        nc.vector.tensor_copy(s_cnuc[:], s_nucT)

        # W quarter 0 immediately (spin0-zeta needs it first; no deps)
        s_w4 = wpool.tile([4, 4 * NORB], f16, tag="w4")
        nc.sync.dma_start(s_w4[:, 0:NORB], d_w4[:, 0:NORB])

        # masked mean-centering of nuclear coords (mask all ones -> count=NN)
        s_mean = const.tile([3, 1], f32, tag="mean")
        nc.vector.tensor_reduce(s_mean[:], s_cnuc[:], AX, AluOpType.add)
        nc.vector.tensor_scalar_mul(s_mean[:], s_mean[:], 1.0 / NN)
        nc.vector.tensor_scalar(s_cnuc[:], s_cnuc[:],
                                s_mean[:, 0:1], None, AluOpType.subtract)

        # pieces for d2[n,e] = |n|^2 + |e|^2 - 2 n.e  (3 accumulating matmuls)
        s_m2n = const.tile([3, NN], f32, tag="m2n")
        nc.vector.tensor_scalar_mul(s_m2n[:], s_nucT, -2.0)
        s_nsq = const.tile([3, NN], f32, tag="nsq")
        nc.vector.tensor_mul(s_nsq[:], s_nucT, s_nucT)
        s_ones3 = const.tile([3, 1], f32, tag="ones3")
        nc.vector.memset(s_ones3[:], 1.0)
        s_onesrow = const.tile([1, NN], f32, tag="onesrow")
        nc.vector.memset(s_onesrow[:], 1.0)

        ps_n2 = ps_tile([1, NN], tag="bk0")
        nc.tensor.matmul(ps_n2[:], lhsT=s_ones3[:], rhs=s_nsq[:],
                         start=True, stop=True)
        s_n2 = const.tile([1, NN], f32, tag="n2")
        nc.vector.tensor_copy(s_n2[:], ps_n2[:])

        s_negd = []
        for s in (0, 1):
            s_esq = const.tile([3, E], f32, tag=f"esq{s}")
            nc.vector.tensor_mul(s_esq[:], s_eT[s], s_eT[s])
            ps_e2 = ps_tile([1, E], tag="bk1")
            nc.tensor.matmul(ps_e2[:], lhsT=s_ones3[:], rhs=s_esq[:],
                             start=True, stop=True)
            s_e2 = const.tile([1, E], f32, tag=f"e2{s}")
            nc.vector.tensor_copy(s_e2[:], ps_e2[:])

            ps_d2 = ps_tile([NN, E], tag="bk2")
            nc.tensor.matmul(ps_d2[:], lhsT=s_m2n[:], rhs=s_eT[s],
                             start=True, stop=False)
            nc.tensor.matmul(ps_d2[:], lhsT=s_n2[:], rhs=s_onesrow[:, 0:E],
                             start=False, stop=False)
            nc.tensor.matmul(ps_d2[:], lhsT=s_onesrow[:], rhs=s_e2[:],
                             start=False, stop=True)
            nd = const.tile([NN, E], f32, tag=f"negd{s}")
            # d = exp(0.5*ln(d2)): stays inside the natural_log_exp table
            # set (sqrt would force a second ACT table load + switch).
            # Guard: the expansion |n|^2+|e|^2-2n.e can round negative for
            # near-coincident points; clamp before Ln.
            s_d2c = const.tile([NN, E], f32, tag=f"d2c{s}")
            nc.vector.tensor_scalar_max(s_d2c[:], ps_d2[:], 1e-24)
            nc.scalar.activation(nd[:], s_d2c[:], AF.Ln)
            nc.scalar.activation(nd[:], nd[:], AF.Exp, scale=0.5)
            nc.vector.tensor_scalar_mul(nd[:], nd[:], -1.0)
            s_negd.append(nd)

        # fp16 feats tile [4, NN] = [charge; centered coords] for K=4 matmuls.
        # Rows 1..3 are placed by DMA (engines cannot write partition base 1).
        s_chg16 = const.tile([1, NN], f16, tag="chg16")
        nc.vector.tensor_copy(s_chg16[:], s_chg)
        s_cnuc16 = const.tile([3, NN], f16, tag="cnuc16")
        nc.vector.tensor_copy(s_cnuc16[:], s_cnuc[:])
        s_f16 = const.tile([4, NN], f16, tag="feats16")
        nc.sync.dma_start(s_f16[0:1, :], s_chg16[:])
        nc.sync.dma_start(s_f16[1:4, :], s_cnuc16[:])
        # remaining W quarters after the feats assembly DMAs
        for q in range(1, 4):
            qs = slice(q * NORB, (q + 1) * NORB)
            nc.sync.dma_start(s_w4[:, qs], d_w4[:, qs])

        # ---------------- zeta / pi ----------------
        s_absz = []
        s_piT = []
        for s in (0, 1):
            s_absz.append(const.tile([128, NORB], f32, tag=f"absz{s}",
                                     name=f"absz{s}"))
            s_piT.append(const.tile([128, NORB], f16, tag=f"pit{s}",
                                    name=f"pit{s}"))

        def w_matmul(dst_ps, w_off, blk):
            sl = slice(w_off + blk * 512, w_off + (blk + 1) * 512)
            nc.tensor.matmul(dst_ps[:], lhsT=s_f16[:], rhs=s_w4[:, sl],
                             start=True, stop=True)

        def emit_zeta_blk(s, blk):
            sl = slice(blk * 512, (blk + 1) * 512)
            ps_z = ps_tile([128, 512])
            w_matmul(ps_z, (2 * s) * NORB, blk)       # zeta uses W_pi
            # |z|: for spin0 alternate engines so the 8-op chain halves in
            # latency (ACT is idle before the exp stream starts); spin1's
            # abs must stay off ACT (it runs during spin0's exp stream)
            if s == 1 or blk % 2 == 0:
                nc.vector.tensor_scalar(
                    s_absz[s][:, sl].bitcast(mybir.dt.uint32),
                    ps_z[:].bitcast(mybir.dt.uint32),
                    0x7FFFFFFF, None, AluOpType.bitwise_and)
            else:
                nc.scalar.activation(s_absz[s][:, sl], ps_z[:], AF.Abs)

        def emit_pi_blk(s, blk):
            sl = slice(blk * 512, (blk + 1) * 512)
            ps_p = ps_tile([128, 512])
            w_matmul(ps_p, (2 * s + 1) * NORB, blk)   # pi uses W_zeta
            nc.vector.tensor_copy(s_piT[s][:, sl], ps_p[:])

        # One-hot is needed before the hoisted first-exp below
        s_oh0 = None

        # spin0's zeta first (needed to start its exp stream), then the rest.
        # The very first exp's halves are emitted INSIDE the zeta loop so the
        # strict ACT FIFO doesn't queue them behind later abs ops.
        t_exp0 = tpool.tile([128, NORB], f16, tag="T", name="texp_e0")
        Hh = NORB // 2
        for blk in range(WBLK):
            emit_zeta_blk(0, blk)
            if blk == 3:
                nc.scalar.activation(t_exp0[:, 0:Hh], s_absz[0][:, 0:Hh],
                                     AF.Exp, scale=s_negd[0][:, 0:1])
        nc.scalar.activation(t_exp0[:, Hh:], s_absz[0][:, Hh:],
                             AF.Exp, scale=s_negd[0][:, 0:1])
        for blk in range(WBLK):
            emit_pi_blk(0, blk)
        for blk in range(WBLK):
            emit_zeta_blk(1, blk)
        for blk in range(WBLK):
            emit_pi_blk(1, blk)

        # One-hot selector: lhsT slice e is [128, E] with column e all-ones.
        # The reduce matmul then computes out[m,o] = sum_n (m==e) * PT[n,o],
        # i.e. the partition-reduction lands on PSUM row e; electrons
        # accumulate into the same [E, 512] bank via start/stop groups.
        # LDWEIGHTS of an [128, E] slice is ~E cycles vs 128 for PT-stationary.
        s_oh = const.tile([128, E * E], f16, tag="onehot")
        nc.vector.memset(s_oh[:], 0.0)
        for e in range(E):
            nc.vector.memset(s_oh[:, e * E + e:e * E + e + 1], 1.0)

        # ---------------- main loop ----------------
        NCHUNK = NORB // 512   # 8 psum banks, one per 512-orbital chunk
        for s in (0, 1):
            ps_orb = [ps_tile([E, 512], tag=f"bk{c}") for c in range(NCHUNK)]
            for e in range(E):
                if s == 0 and e == 0:
                    t_exp = t_exp0   # activations already emitted (hoisted)
                elif s == 1 and (e == 0 or e == E - 1):
                    # split spin1's first exp (smooths the spin boundary) and
                    # the last exp (tail TT/matmuls/evac start half earlier)
                    t_exp = tpool.tile([128, NORB], f16, tag="T")
                    Hh = NORB // 2
                    nc.scalar.activation(t_exp[:, 0:Hh],
                                         s_absz[s][:, 0:Hh], AF.Exp,
                                         scale=s_negd[s][:, e:e + 1])
                    nc.scalar.activation(t_exp[:, Hh:],
                                         s_absz[s][:, Hh:], AF.Exp,
                                         scale=s_negd[s][:, e:e + 1])
                else:
                    t_exp = tpool.tile([128, NORB], f16, tag="T")
                    nc.scalar.activation(t_exp[:], s_absz[s][:], AF.Exp,
                                         scale=s_negd[s][:, e:e + 1])
                # in-place pi-weighting: t_exp *= piT (quarters so the
                # first reduce matmuls start after 1/4 of the multiply)
                H = NORB // 4
                for h in range(4):
                    nc.vector.tensor_mul(t_exp[:, h * H:(h + 1) * H],
                                         t_exp[:, h * H:(h + 1) * H],
                                         s_piT[s][:, h * H:(h + 1) * H])
                for c in range(NCHUNK):
                    nc.tensor.matmul(ps_orb[c][:],
                                     lhsT=s_oh[:, e * E:(e + 1) * E],
                                     rhs=t_exp[:, c * 512:(c + 1) * 512],
                                     start=(e == 0), stop=(e == E - 1))
            for c in range(NCHUNK):
                s_o = opool.tile([E, 512], f32, tag="osb")
                if c % 2 == 0:
                    nc.vector.tensor_copy(s_o[:], ps_orb[c][:])
                else:
                    nc.scalar.copy(s_o[:], ps_orb[c][:])
                # spread the final evac DMAs across two queues
                dma_eng = nc.gpsimd if (s == 1 and c % 2 == 1) else nc.sync
                dma_eng.dma_start(d_out[s][:, c * 512:(c + 1) * 512], s_o[:])

    return nc


def _get_module():
    if "nc" not in _CACHE:
        _CACHE["nc"] = _build_module()
    return _CACHE["nc"]


def kernel(**inputs) -> np.ndarray:
    global LAST_RESULTS
    nc = _get_module()
    from concourse.bass_utils import run_bass_kernel_spmd

    up = np.ascontiguousarray(np.asarray(inputs["up_coords"], dtype=np.float32))
    down = np.ascontiguousarray(np.asarray(inputs["down_coords"], dtype=np.float32))
    nuc = np.asarray(inputs["nuc_coords"], dtype=np.float32)
    chg = np.asarray(inputs["nuc_charges"], dtype=np.float32)
    w = {
        k: np.ascontiguousarray(np.asarray(inputs[k], dtype=np.float32))
        for k in ("W_pi_up", "W_zeta_up", "W_pi_down", "W_zeta_down")
    }
    nucT = nuc.T                                  # [3, 128]

    worder = ("W_pi_up", "W_zeta_up", "W_pi_down", "W_zeta_down")
    wsplit = {
        "w4": np.ascontiguousarray(np.concatenate(
            [w[n] for n in worder], axis=1).astype(np.float16)),
    }

    in_maps = []
    for c in range(N_CORES):
        sl = slice(c * E_PER_CORE, (c + 1) * E_PER_CORE)
        small = np.zeros((3, 2 * NN + 2 * E_PER_CORE), dtype=np.float32)
        small[:, 0:NN] = nucT
        small[0, NN:2 * NN] = chg
        small[:, 2 * NN:2 * NN + E_PER_CORE] = up[sl].T
        small[:, 2 * NN + E_PER_CORE:] = down[sl].T
        in_maps.append({"small": small, **wsplit})

    res = run_bass_kernel_spmd(nc, in_maps, core_ids=list(range(N_CORES)))
    LAST_RESULTS = res

    # gather: per-core slab is already [2, e_local, orbital]
    orb = np.empty((2, NE, NORB), dtype=np.float32)
    for c in range(N_CORES):
        a = np.asarray(res.results[c]["out"])            # [2, E, NORB]
        orb[:, c * E_PER_CORE:(c + 1) * E_PER_CORE, :] = a

    # [2, n_e, n_det*max_e] -> [2, n_det, n_e, max_e]
    out = orb.reshape(2, NE, NDET, NE).swapaxes(1, 2)
    return np.ascontiguousarray(out)



# revision 8
# speedup vs baseline: 1.0852x; 1.0852x over previous
"""Trainium2 Bass kernel for the ExponentialEnvelopes module.

Math (per spin):
    feats[n,k]  = [charge, centered coords]           (nuclei features, [128, 4])
    Z[n,o]      = (feats @ W_pi)[n,o]                 (= zeta.T)
    P[n,o]      = (feats @ W_zeta)[n,o]               (= pi.T)
    d[e,n]      = ||e_coords[e] - nuc_coords[n]||
    orb[e,o]    = sum_n P[n,o] * exp(-d[e,n] * |Z[n,o]|)

Hybrid exp strategy (ACT is the scalar-engine LUT bottleneck at ~3.7us per
[128,4096] exp):
  * ACT path (A electrons): X16 = fp16(1024*log2e*|Z|) (W_pi host-prescaled
    by 1477.32), ACT Exp with per-partition scale = -d*ln2/1024, then DVE
    fp16 tensor_tensor multiply by piT.
  * DVE path (B electrons): int16 Schraudolph fast exp on the vector engine:
      i16 = int16(X16 * (-d) + (15360 - c - 32768))   [tensor_scalar, fp16 in]
    The -32768 shift makes every valid result land in [-32768, -2048] as a
    *negative-encoded* fp16 magnitude (bitcast): sign bit set, exponent field
    <= 30 (never Inf/NaN), and deep underflow saturates to -32768 = -0.0.
    Multiply by piT gives -pi*exp; the one-hot reduction column for these
    electrons is -1 so PSUM accumulates +pi*exp.  Max rel err of the sawtooth
    is ~3% per term, zero-mean at c=60, and the final output error stays
    ~1.5e-2 relative to global max (tolerance 2e-2) even all-DVE.
  * PSUM evacuations (X16 abs, piT copy, orb) are done in 4-bank [*, 2048]
    chunks, mostly on ScE (cheaper per element than DVE and off the critical
    DVE path).
  * Lean exit: single-semaphore gather barrier + sem clears instead of the
    stock drain + 2 butterfly barriers (saves ~6us of EVENT_SEMAPHORE storm).

Sharding: electrons across the 8 cores (16/core, both spins), orbitals whole.
"""

import numpy as np
from contextlib import ExitStack

NE = 128          # electrons per spin (total)
NN = 128          # nuclei
NDET = 32
NORB = 4096       # n_det * max_e
N_CORES = 8
E_PER_CORE = NE // N_CORES   # 16

# Electrons computed via the DVE fast-exp path, per spin (rest on ACT).
DVE_E = [
    frozenset({1, 3, 5, 7, 9, 11, 13}),      # spin 0: 7 DVE / 9 ACT
    frozenset({1, 3, 5, 7, 9, 11, 13, 14}),  # spin 1: 8 DVE / 8 ACT
]
C_FOLD = 1477.3196  # 1024*log2(e); host multiplies W_pi by this
LN2_1024 = 6.7711243e-4  # ln2/1024: ACT scale factor on -d
SCHRAUD_C = 60.0
SCHRAUD_S2 = -17408.0 - SCHRAUD_C   # 15360 - c - 32768

_CACHE = {}

LAST_RESULTS = None  # BassKernelResults of the most recent run (for test harness)


def _split_multiwaits(nc, blocks):
    """Every TPB engine instruction has exactly ONE embedded sync-wait slot;
    Tile's sem assignment can emit several waits on one instruction, which
    walrus rejects.  Hoist all but the last wait onto fresh single-wait NOPs
    inserted just before the instruction on the same engine stream."""
    from concourse import mybir

    for bb, insts in blocks.items():
        out = []
        changed = False
        for inst in insts:
            si = getattr(inst, "sync_info", None)
            waits = list(si.on_wait) if si is not None and si.on_wait else []
            if len(waits) > 1:
                for w in waits[:-1]:
                    nop = mybir.InstNoOp(
                        name=nc.get_next_instruction_name(), ins=[], outs=[])
                    nop.engine = inst.engine
                    nop.sync_info = mybir.SyncInfo(on_wait=[w], on_update=[])
                    out.append(nop)
                inst.sync_info = mybir.SyncInfo(
                    on_wait=[waits[-1]], on_update=list(si.on_update))
                changed = True
            out.append(inst)
        if changed:
            insts[:] = out


def _build_module():
    import concourse.bass as bass
    import concourse.tile as tile
    from concourse import mybir
    from concourse.alu_op_type import AluOpType

    class FixupTileContext(tile.TileContext):
        def _lower_ordered_insts(self, postordered_blocks):
            _split_multiwaits(self.nc, postordered_blocks)
            return super()._lower_ordered_insts(postordered_blocks)

        def _drain_and_barrier(self, tick_clock, wait_clock):
            # Pre-observe the full global clock on the sync engine via
            # single-wait NOPs (one embedded wait slot per instruction), then
            # drain the DMA queues.  Replaces the stock exit (drain + two
            # butterfly all-engine barriers around the sem clears) with a
            # single-semaphore gather: each engine incs once as its final
            # instruction; gpsimd waits for all and clears the sems.
            from concourse.vector_clock import ScopedClock

            probe = self.nc.sync.nop()
            wait_clock.add_sem_waits(
                probe.ins, ScopedClock({None: tick_clock.global_clock}))
            si = probe.ins.sync_info
            waits = list(si.on_wait) if si is not None and si.on_wait else []
            if len(waits) > 1:
                probe.ins.sync_info = mybir.SyncInfo(
                    on_wait=[waits[0]], on_update=list(si.on_update or []))
                for w in waits[1:]:
                    extra = self.nc.sync.nop()
                    extra.ins.sync_info = mybir.SyncInfo(
                        on_wait=[w], on_update=[])
            self.nc.sync.drain()

            exit_sem = self.nc.alloc_semaphore("lean_exit")
            self.nc.tensor.sem_inc(exit_sem, 1)
            self.nc.scalar.sem_inc(exit_sem, 1)
            self.nc.vector.sem_inc(exit_sem, 1)
            self.nc.sync.sem_inc(exit_sem, 1)
            self.nc.gpsimd.wait_ge(exit_sem, 4)
            popped = self.nc._tile_sem_poison_stack.pop()
            assert popped is self._sem_poison
            self.nc.clear_and_free_semaphores(
                list(self.sems.allocated().values()) + [exit_sem])

    f32 = mybir.dt.float32
    f16 = mybir.dt.float16
    i16 = mybir.dt.int16
    AF = mybir.ActivationFunctionType
    AX = mybir.AxisListType.X
    E = E_PER_CORE

    nc = bass.Bass(trn_type="TRN2")

    # all small inputs packed into one DMA: [3, 288] =
    #   [:, 0:128] nucT rows, [0, 128:256] charges, [:, 256:272] eT_up,
    #   [:, 272:288] eT_dn  (all slices start at partition 0)
    d_small = nc.dram_tensor("small", [3, 2 * NN + 2 * E], f32,
                             kind="ExternalInput")
    # W matrices pre-split by the host into charge rows (k=0) and coord rows
    # (k=1..3) so every SBUF access pattern starts at partition 0; all four
    # matrices are packed along the free dim: index (s, m) at (2*s+m)*NORB.
    # The zeta matrices (m=0) are host-premultiplied by C_FOLD.
    d_w4 = nc.dram_tensor("w4", [4, 4 * NORB], f16, kind="ExternalInput")
    # per-core output slab: [spin][e_local][orbital] (directly in orb layout)
    d_out = nc.dram_tensor("out", [2, E, NORB], f32, kind="ExternalOutput")

    with ExitStack() as ctx:
        tc = ctx.enter_context(FixupTileContext(nc))
        const = ctx.enter_context(tc.tile_pool(name="const", bufs=1))
        wpool = ctx.enter_context(tc.tile_pool(name="wload", bufs=1))
        tpool = ctx.enter_context(tc.tile_pool(name="texp", bufs=6))
        opool = ctx.enter_context(tc.tile_pool(name="outsb", bufs=4))
        psum = ctx.enter_context(tc.tile_pool(name="ps", bufs=1, space="PSUM"))
        _id = [0]

        def ps_tile(shape, tag):
            _id[0] += 1
            return psum.tile(shape, f32, tag=tag, name=f"ps{_id[0]}_{tag}")

        # ---------------- small loads (single DMA) ----------------
        s_small = const.tile([3, 2 * NN + 2 * E], f32, tag="small")
        nc.sync.dma_start(s_small[:], d_small[:])
        s_nucT = s_small[:, 0:NN]
        s_chg = s_small[0:1, NN:2 * NN]
        s_eT = [s_small[:, 2 * NN:2 * NN + E],
                s_small[:, 2 * NN + E:2 * NN + 2 * E]]
        s_cnuc = const.tile([3, NN], f32, tag="cnuc")  # centered coords
        nc.vector.tensor_copy(s_cnuc[:], s_nucT)

        # W quarter 0 immediately (spin0-zeta needs it first; no deps)
        s_w4 = wpool.tile([4, 4 * NORB], f16, tag="w4")
        nc.sync.dma_start(s_w4[:, 0:NORB], d_w4[:, 0:NORB])

        # masked mean-centering of nuclear coords (mask all ones -> count=NN)
        s_mean = const.tile([3, 1], f32, tag="mean")
        nc.vector.tensor_reduce(s_mean[:], s_cnuc[:], AX, AluOpType.add)
        nc.vector.tensor_scalar_mul(s_mean[:], s_mean[:], 1.0 / NN)
        nc.vector.tensor_scalar(s_cnuc[:], s_cnuc[:],
                                s_mean[:, 0:1], None, AluOpType.subtract)

        # pieces for d2[n,e] = |n|^2 + |e|^2 - 2 n.e  (3 accumulating matmuls)
        s_m2n = const.tile([3, NN], f32, tag="m2n")
        nc.vector.tensor_scalar_mul(s_m2n[:], s_nucT, -2.0)
        s_nsq = const.tile([3, NN], f32, tag="nsq")
        nc.vector.tensor_mul(s_nsq[:], s_nucT, s_nucT)
        s_ones3 = const.tile([3, 1], f32, tag="ones3")
        nc.vector.memset(s_ones3[:], 1.0)
        s_onesrow = const.tile([1, NN], f32, tag="onesrow")
        nc.vector.memset(s_onesrow[:], 1.0)

        ps_n2 = ps_tile([1, NN], tag="wA")
        nc.tensor.matmul(ps_n2[:], lhsT=s_ones3[:], rhs=s_nsq[:],
                         start=True, stop=True)
        s_n2 = const.tile([1, NN], f32, tag="n2")
        nc.vector.tensor_copy(s_n2[:], ps_n2[:])

        # negd[s]   : [NN, E] = -d          (DVE-path tensor_scalar scale)
        # negd_a[s] : [NN, E] = -d*ln2/1024 (ACT Exp scale against X16)
        s_negd = []
        s_negd_a = []
        for s in (0, 1):
            s_esq = const.tile([3, E], f32, tag=f"esq{s}")
            nc.vector.tensor_mul(s_esq[:], s_eT[s], s_eT[s])
            ps_e2 = ps_tile([1, E], tag="wA")
            nc.tensor.matmul(ps_e2[:], lhsT=s_ones3[:], rhs=s_esq[:],
                             start=True, stop=True)
            s_e2 = const.tile([1, E], f32, tag=f"e2{s}")
            nc.vector.tensor_copy(s_e2[:], ps_e2[:])

            ps_d2 = ps_tile([NN, E], tag="wA")
            nc.tensor.matmul(ps_d2[:], lhsT=s_m2n[:], rhs=s_eT[s],
                             start=True, stop=False)
            nc.tensor.matmul(ps_d2[:], lhsT=s_n2[:], rhs=s_onesrow[:, 0:E],
                             start=False, stop=False)
            nc.tensor.matmul(ps_d2[:], lhsT=s_onesrow[:], rhs=s_e2[:],
                             start=False, stop=True)
            nd = const.tile([NN, E], f32, tag=f"negd{s}")
            # d = exp(0.5*ln(d2)): stays inside the natural_log_exp table
            # set (sqrt would force a second ACT table load + switch).
            # Guard: the expansion |n|^2+|e|^2-2n.e can round negative for
            # near-coincident points; clamp before Ln.
            s_d2c = const.tile([NN, E], f32, tag=f"d2c{s}")
            nc.vector.tensor_scalar_max(s_d2c[:], ps_d2[:], 1e-24)
            nc.scalar.activation(nd[:], s_d2c[:], AF.Ln)
            nc.scalar.activation(nd[:], nd[:], AF.Exp, scale=0.5)
            nc.vector.tensor_scalar_mul(nd[:], nd[:], -1.0)
            nda = const.tile([NN, E], f32, tag=f"negda{s}")
            nc.vector.tensor_scalar_mul(nda[:], nd[:], LN2_1024)
            s_negd.append(nd)
            s_negd_a.append(nda)

        # fp16 feats tile [4, NN] = [charge; centered coords] for K=4 matmuls.
        # Rows 1..3 are placed by DMA (engines cannot write partition base 1).
        s_chg16 = const.tile([1, NN], f16, tag="chg16")
        nc.vector.tensor_copy(s_chg16[:], s_chg)
        s_cnuc16 = const.tile([3, NN], f16, tag="cnuc16")
        nc.vector.tensor_copy(s_cnuc16[:], s_cnuc[:])
        s_f16 = const.tile([4, NN], f16, tag="feats16")
        nc.sync.dma_start(s_f16[0:1, :], s_chg16[:])
        nc.sync.dma_start(s_f16[1:4, :], s_cnuc16[:])
        # remaining W quarters after the feats assembly DMAs
        for q in range(1, 4):
            qs = slice(q * NORB, (q + 1) * NORB)
            nc.sync.dma_start(s_w4[:, qs], d_w4[:, qs])

        # One-hot selector per (spin, electron): lhsT slice (s,e) is [128, E]
        # with column e = +1 (ACT path) or -1 (DVE path, negated encoding).
        # memsets on gpsimd (free engine).
        s_oh = const.tile([128, 2 * E * E], f16, tag="onehot")
        nc.gpsimd.memset(s_oh[:], 0.0)
        for s in (0, 1):
            for e in range(E):
                col = (s * E + e) * E + e
                val = -1.0 if e in DVE_E[s] else 1.0
                nc.gpsimd.memset(s_oh[:, col:col + 1], val)

        def oh(s, e):
            base = (s * E + e) * E
            return s_oh[:, base:base + E]

        # ---------------- zeta / pi -> X16 / piT ----------------
        # X16[s] = fp16(C_FOLD*|z|)   piT[s] = fp16(pi)
        # PSUM: two 4-bank [128, 2048] tiles (tags wA / wB); each W matrix
        # needs two of them (NORB=4096).  Sequence per bank-group is
        # serialized by the evacuation reads.
        s_X16 = []
        s_piT = []
        for s in (0, 1):
            s_X16.append(const.tile([128, NORB], f16, tag=f"x16_{s}",
                                    name=f"x16_{s}"))
            s_piT.append(const.tile([128, NORB], f16, tag=f"pit{s}",
                                    name=f"pit{s}"))

        HW = 2048  # evac half-width (4 PSUM banks)

        def w_group(s, m, half):
            """Matmuls for W matrix (s, m) covering orbital cols
            [half*2048, (half+1)*2048) into a fresh 4-bank PSUM tile."""
            tag = "wA" if (2 * s + m + half) % 2 == 0 else "wB"
            ps = ps_tile([128, HW], tag=tag)
            w_off = (2 * s + m) * NORB + half * HW
            for q in range(4):
                sl = slice(w_off + q * 512, w_off + (q + 1) * 512)
                nc.tensor.matmul(ps[:, q * 512:(q + 1) * 512],
                                 lhsT=s_f16[:], rhs=s_w4[:, sl],
                                 start=True, stop=True)
            return ps

        def evac_x16(s, half, ps, eng):
            # abs_max is not a valid tensor_scalar ALU op (walrus ISA check),
            # so all X16 evacs go through ScE AF.Abs regardless of `eng`.
            dst = s_X16[s][:, half * HW:(half + 1) * HW]
            nc.scalar.activation(dst, ps[:], AF.Abs)

        def evac_pit(s, half, ps, eng):
            dst = s_piT[s][:, half * HW:(half + 1) * HW]
            if eng == "v":
                nc.vector.tensor_copy(dst, ps[:])
            else:
                nc.scalar.copy(dst, ps[:])

        # ---------------- electron emission helpers ----------------
        # orb accumulators: two 4-bank [E, 2048] tiles per spin (tags wA/wB,
        # recycled from the W tiles once those are evacuated).
        NCHUNK = NORB // 512

        def emit_exp_act(s, e, t_dst, halves):
            if halves:
                nc.scalar.activation(t_dst[:, 0:HW], s_X16[s][:, 0:HW],
                                     AF.Exp, scale=s_negd_a[s][:, e:e + 1])
                nc.scalar.activation(t_dst[:, HW:], s_X16[s][:, HW:],
                                     AF.Exp, scale=s_negd_a[s][:, e:e + 1])
            else:
                nc.scalar.activation(t_dst[:], s_X16[s][:], AF.Exp,
                                     scale=s_negd_a[s][:, e:e + 1])

        def emit_fastexp_dve(s, e, t_i):
            nc.vector.tensor_scalar(t_i[:], s_X16[s][:],
                                    s_negd[s][:, e:e + 1], SCHRAUD_S2,
                                    AluOpType.mult, AluOpType.add)

        def emit_pimul(s, t_f):
            # in-place fp16 multiply by piT, halves for PE pipelining
            for h in (0, 1):
                sl = slice(h * HW, (h + 1) * HW)
                nc.vector.tensor_mul(t_f[:, sl], t_f[:, sl], s_piT[s][:, sl])

        def emit_mms(s, e, t_f, ps_orb):
            first = e == 0
            last = e == E - 1
            for c in range(NCHUNK):
                dst = ps_orb[c // 4][:, (c % 4) * 512:(c % 4 + 1) * 512]
                nc.tensor.matmul(dst, lhsT=oh(s, e),
                                 rhs=t_f[:, c * 512:(c + 1) * 512],
                                 start=first, stop=last)

        def emit_electron(s, e, ps_orb, halves=False):
            if e in DVE_E[s]:
                t_i = tpool.tile([128, NORB], i16, tag="T")
                emit_fastexp_dve(s, e, t_i)
                t_f = t_i[:].bitcast(f16)
                emit_pimul(s, t_f)
                emit_mms(s, e, t_f, ps_orb)
            else:
                t_e = tpool.tile([128, NORB], f16, tag="T")
                emit_exp_act(s, e, t_e, halves)
                emit_pimul(s, t_e[:])
                emit_mms(s, e, t_e[:], ps_orb)

        # --- setup emission, interleaved with spin0's first electrons so no
        # engine starves.  X16 spin0 split DVE/ACT for head latency; the rest
        # mostly on ScE (off the hot DVE path).
        ps = w_group(0, 0, 0)
        evac_x16(0, 0, ps, "v")
        t_e0 = tpool.tile([128, NORB], f16, tag="T")
        nc.scalar.activation(t_e0[:, 0:HW], s_X16[0][:, 0:HW],
                             AF.Exp, scale=s_negd_a[0][:, 0:1])
        ps = w_group(0, 0, 1)
        evac_x16(0, 1, ps, "a")
        nc.scalar.activation(t_e0[:, HW:], s_X16[0][:, HW:],
                             AF.Exp, scale=s_negd_a[0][:, 0:1])
        ps = w_group(0, 1, 0)
        evac_pit(0, 0, ps, "v")
        ps = w_group(0, 1, 1)
        evac_pit(0, 1, ps, "a")
        emit_pimul(0, t_e0[:])

        # spin1 setup (X16 on DVE early, piT on ACT), interleaved with the
        # first spin0 electrons
        ps_orb0 = [psum.tile([E, HW], f32, tag="wA", name="orb0A"),
                   psum.tile([E, HW], f32, tag="wB", name="orb0B")]
        emit_mms(0, 0, t_e0[:], ps_orb0)
        emit_electron(0, 1, ps_orb0)
        ps = w_group(1, 0, 0)
        evac_x16(1, 0, ps, "v")
        emit_electron(0, 2, ps_orb0)
        ps = w_group(1, 0, 1)
        evac_x16(1, 1, ps, "v")
        emit_electron(0, 3, ps_orb0)
        ps = w_group(1, 1, 0)
        evac_pit(1, 0, ps, "a")
        emit_electron(0, 4, ps_orb0)
        ps = w_group(1, 1, 1)
        evac_pit(1, 1, ps, "a")
        for e in range(5, E):
            emit_electron(0, e, ps_orb0)

        # spin0 orb evacuation ([E, 2048] x2, one per engine) + DMA out
        s_o0 = [opool.tile([E, HW], f32, tag="osb", name=f"o0_{i}")
                for i in range(2)]
        nc.vector.tensor_copy(s_o0[0][:], ps_orb0[0][:])
        nc.scalar.copy(s_o0[1][:], ps_orb0[1][:])
        nc.sync.dma_start(d_out[0][:, 0:HW], s_o0[0][:])
        nc.gpsimd.dma_start(d_out[0][:, HW:], s_o0[1][:])

        # ---------------- spin 1 main loop ----------------
        ps_orb1 = [psum.tile([E, HW], f32, tag="wA", name="orb1A"),
                   psum.tile([E, HW], f32, tag="wB", name="orb1B")]
        for e in range(E):
            emit_electron(1, e, ps_orb1, halves=(e in (0, E - 1)))

        s_o1 = [opool.tile([E, HW], f32, tag="osb", name=f"o1_{i}")
                for i in range(2)]
        nc.vector.tensor_copy(s_o1[0][:], ps_orb1[0][:])
        nc.scalar.copy(s_o1[1][:], ps_orb1[1][:])
        nc.sync.dma_start(d_out[1][:, 0:HW], s_o1[0][:])
        nc.gpsimd.dma_start(d_out[1][:, HW:], s_o1[1][:])

    return nc


def _get_module():
    if "nc" not in _CACHE:
        _CACHE["nc"] = _build_module()
    return _CACHE["nc"]


def kernel(**inputs) -> np.ndarray:
    global LAST_RESULTS
    nc = _get_module()
    from concourse.bass_utils import run_bass_kernel_spmd

    up = np.ascontiguousarray(np.asarray(inputs["up_coords"], dtype=np.float32))
    down = np.ascontiguousarray(np.asarray(inputs["down_coords"], dtype=np.float32))
    nuc = np.asarray(inputs["nuc_coords"], dtype=np.float32)
    chg = np.asarray(inputs["nuc_charges"], dtype=np.float32)
    w = {
        k: np.ascontiguousarray(np.asarray(inputs[k], dtype=np.float32))
        for k in ("W_pi_up", "W_zeta_up", "W_pi_down", "W_zeta_down")
    }
    nucT = nuc.T                                  # [3, 128]

    # zeta sources (W_pi_*) host-prescaled by C_FOLD for the fp16 X16 layout
    wmats = [w["W_pi_up"] * np.float32(C_FOLD), w["W_zeta_up"],
             w["W_pi_down"] * np.float32(C_FOLD), w["W_zeta_down"]]
    wsplit = {
        "w4": np.ascontiguousarray(
            np.concatenate(wmats, axis=1).astype(np.float16)),
    }

    in_maps = []
    for c in range(N_CORES):
        sl = slice(c * E_PER_CORE, (c + 1) * E_PER_CORE)
        small = np.zeros((3, 2 * NN + 2 * E_PER_CORE), dtype=np.float32)
        small[:, 0:NN] = nucT
        small[0, NN:2 * NN] = chg
        small[:, 2 * NN:2 * NN + E_PER_CORE] = up[sl].T
        small[:, 2 * NN + E_PER_CORE:] = down[sl].T
        in_maps.append({"small": small, **wsplit})

    res = run_bass_kernel_spmd(nc, in_maps, core_ids=list(range(N_CORES)))
    LAST_RESULTS = res

    # gather: per-core slab is already [2, e_local, orbital]
    orb = np.empty((2, NE, NORB), dtype=np.float32)
    for c in range(N_CORES):
        a = np.asarray(res.results[c]["out"])            # [2, E, NORB]
        orb[:, c * E_PER_CORE:(c + 1) * E_PER_CORE, :] = a

    # [2, n_e, n_det*max_e] -> [2, n_det, n_e, max_e]
    out = orb.reshape(2, NE, NDET, NE).swapaxes(1, 2)
    return np.ascontiguousarray(out)


# revision 15
# speedup vs baseline: 1.1146x; 1.0272x over previous
"""Trainium2 Bass kernel for the ExponentialEnvelopes module.

Math (per spin):
    feats[n,k]  = [charge, centered coords]           (nuclei features, [128, 4])
    Z[n,o]      = (feats @ W_pi)[n,o]                 (= zeta.T)
    P[n,o]      = (feats @ W_zeta)[n,o]               (= pi.T)
    d[e,n]      = ||e_coords[e] - nuc_coords[n]||
    orb[e,o]    = sum_n P[n,o] * exp(-d[e,n] * |Z[n,o]|)

Hybrid exp strategy (ACT is the scalar-engine LUT bottleneck at ~3.7us per
[128,4096] exp):
  * ACT path (A electrons): X16 = fp16(1024*log2e*|Z|) (W_pi host-prescaled
    by 1477.32), ACT Exp with per-partition scale = -d*ln2/1024, then DVE
    fp16 tensor_tensor multiply by piT.
  * DVE path (B electrons): int16 Schraudolph fast exp on the vector engine:
      i16 = int16(X16 * (-d) + (15360 - c - 32768))   [tensor_scalar, fp16 in]
    The -32768 shift makes every valid result land in [-32768, -2048] as a
    *negative-encoded* fp16 magnitude (bitcast): sign bit set, exponent field
    <= 30 (never Inf/NaN), and deep underflow saturates to -32768 = -0.0.
    Multiply by piT gives -pi*exp; the one-hot reduction column for these
    electrons is -1 so PSUM accumulates +pi*exp.  Max rel err of the sawtooth
    is ~3% per term, zero-mean at c=60, and the final output error stays
    ~1.5e-2 relative to global max (tolerance 2e-2) even all-DVE.
  * PSUM evacuations (X16 abs, piT copy, orb) are done in 4-bank [*, 2048]
    chunks, mostly on ScE (cheaper per element than DVE and off the critical
    DVE path).
  * Lean exit: single-semaphore gather barrier + sem clears instead of the
    stock drain + 2 butterfly barriers (saves ~6us of EVENT_SEMAPHORE storm).

Sharding: electrons across the 8 cores (16/core, both spins), orbitals whole.
"""

import numpy as np
from contextlib import ExitStack

NE = 128          # electrons per spin (total)
NN = 128          # nuclei
NDET = 32
NORB = 4096       # n_det * max_e
N_CORES = 8
E_PER_CORE = NE // N_CORES   # 16

# Electrons computed via the DVE fast-exp path, per spin (rest on ACT).
DVE_E = [
    frozenset({1, 3, 5, 7, 9, 11}),          # spin 0: 6 DVE / 10 ACT
    frozenset({1, 3, 5, 7, 9, 11, 13}),      # spin 1: 7 DVE / 9 ACT
]
C_FOLD = 1477.3196  # 1024*log2(e); host multiplies W_pi by this
LN2_1024 = 6.7711243e-4  # ln2/1024: ACT scale factor on -d
SCHRAUD_C = 60.0
SCHRAUD_S2 = -17408.0 - SCHRAUD_C   # 15360 - c - 32768

_CACHE = {}

LAST_RESULTS = None  # BassKernelResults of the most recent run (for test harness)


def _split_multiwaits(nc, blocks):
    """Every TPB engine instruction has exactly ONE embedded sync-wait slot;
    Tile's sem assignment can emit several waits on one instruction, which
    walrus rejects.  Hoist all but the last wait onto fresh single-wait NOPs
    inserted just before the instruction on the same engine stream."""
    from concourse import mybir

    for bb, insts in blocks.items():
        out = []
        changed = False
        for inst in insts:
            si = getattr(inst, "sync_info", None)
            waits = list(si.on_wait) if si is not None and si.on_wait else []
            if len(waits) > 1:
                for w in waits[:-1]:
                    nop = mybir.InstNoOp(
                        name=nc.get_next_instruction_name(), ins=[], outs=[])
                    nop.engine = inst.engine
                    nop.sync_info = mybir.SyncInfo(on_wait=[w], on_update=[])
                    out.append(nop)
                inst.sync_info = mybir.SyncInfo(
                    on_wait=[waits[-1]], on_update=list(si.on_update))
                changed = True
            out.append(inst)
        if changed:
            insts[:] = out


def _build_module():
    import concourse.bass as bass
    import concourse.tile as tile
    from concourse import mybir
    from concourse.alu_op_type import AluOpType

    class FixupTileContext(tile.TileContext):
        def _lower_ordered_insts(self, postordered_blocks):
            _split_multiwaits(self.nc, postordered_blocks)
            return super()._lower_ordered_insts(postordered_blocks)

        def _drain_and_barrier(self, tick_clock, wait_clock):
            # Pre-observe the full global clock on the sync engine via
            # single-wait NOPs (one embedded wait slot per instruction), then
            # drain the DMA queues.  Replaces the stock exit (drain + two
            # butterfly all-engine barriers around the sem clears) with a
            # single-semaphore gather: each engine incs once as its final
            # instruction; gpsimd waits for all and clears the sems.
            from concourse.vector_clock import ScopedClock

            probe = self.nc.sync.nop()
            wait_clock.add_sem_waits(
                probe.ins, ScopedClock({None: tick_clock.global_clock}))
            si = probe.ins.sync_info
            waits = list(si.on_wait) if si is not None and si.on_wait else []
            if len(waits) > 1:
                probe.ins.sync_info = mybir.SyncInfo(
                    on_wait=[waits[0]], on_update=list(si.on_update or []))
                for w in waits[1:]:
                    extra = self.nc.sync.nop()
                    extra.ins.sync_info = mybir.SyncInfo(
                        on_wait=[w], on_update=[])
            self.nc.sync.drain()

            exit_sem = self.nc.alloc_semaphore("lean_exit")
            self.nc.tensor.sem_inc(exit_sem, 1)
            self.nc.scalar.sem_inc(exit_sem, 1)
            self.nc.vector.sem_inc(exit_sem, 1)
            self.nc.sync.sem_inc(exit_sem, 1)
            self.nc.gpsimd.wait_ge(exit_sem, 4)
            popped = self.nc._tile_sem_poison_stack.pop()
            assert popped is self._sem_poison
            self.nc.clear_and_free_semaphores(
                list(self.sems.allocated().values()) + [exit_sem])

    f32 = mybir.dt.float32
    f16 = mybir.dt.float16
    i16 = mybir.dt.int16
    AF = mybir.ActivationFunctionType
    AX = mybir.AxisListType.X
    E = E_PER_CORE

    nc = bass.Bass(trn_type="TRN2")

    # all small inputs packed into one DMA: [3, 288] =
    #   [:, 0:128] nucT rows, [0, 128:256] charges, [:, 256:272] eT_up,
    #   [:, 272:288] eT_dn  (all slices start at partition 0)
    d_small = nc.dram_tensor("small", [3, 2 * NN + 2 * E], f32,
                             kind="ExternalInput")
    # W matrices pre-split by the host into charge rows (k=0) and coord rows
    # (k=1..3) so every SBUF access pattern starts at partition 0; all four
    # matrices are packed along the free dim: index (s, m) at (2*s+m)*NORB.
    # The zeta matrices (m=0) are host-premultiplied by C_FOLD.
    d_w4 = nc.dram_tensor("w4", [4, 4 * NORB], f16, kind="ExternalInput")
    # per-core output slab: [spin][e_local][orbital] (directly in orb layout)
    # fp16: halves the output DMA; |orb| <= ~200 so range is fine and the
    # 2^-11 quantization is far below the fast-exp error already accepted
    d_out = nc.dram_tensor("out", [2, E, NORB], f16, kind="ExternalOutput")

    with ExitStack() as ctx:
        tc = ctx.enter_context(FixupTileContext(nc))
        const = ctx.enter_context(tc.tile_pool(name="const", bufs=1))
        wpool = ctx.enter_context(tc.tile_pool(name="wload", bufs=1))
        tpool = ctx.enter_context(tc.tile_pool(name="texp", bufs=10))
        opool = ctx.enter_context(tc.tile_pool(name="outsb", bufs=4))
        psum = ctx.enter_context(tc.tile_pool(name="ps", bufs=1, space="PSUM"))
        _id = [0]

        def ps_tile(shape, tag):
            _id[0] += 1
            return psum.tile(shape, f32, tag=tag, name=f"ps{_id[0]}_{tag}")

        # ---------------- small loads (single DMA) ----------------
        s_small = const.tile([3, 2 * NN + 2 * E], f32, tag="small")
        nc.sync.dma_start(s_small[:], d_small[:])
        s_nucT = s_small[:, 0:NN]
        s_chg = s_small[0:1, NN:2 * NN]
        s_eT = [s_small[:, 2 * NN:2 * NN + E],
                s_small[:, 2 * NN + E:2 * NN + 2 * E]]
        s_cnuc = const.tile([3, NN], f32, tag="cnuc")  # centered coords
        nc.vector.tensor_copy(s_cnuc[:], s_nucT)

        # W quarter 0 immediately (spin0-zeta needs it first; no deps)
        s_w4 = wpool.tile([4, 4 * NORB], f16, tag="w4")
        nc.sync.dma_start(s_w4[:, 0:NORB], d_w4[:, 0:NORB])

        # masked mean-centering of nuclear coords (mask all ones -> count=NN)
        s_mean = const.tile([3, 1], f32, tag="mean")
        nc.vector.tensor_reduce(s_mean[:], s_cnuc[:], AX, AluOpType.add)
        nc.vector.tensor_scalar_mul(s_mean[:], s_mean[:], 1.0 / NN)
        nc.vector.tensor_scalar(s_cnuc[:], s_cnuc[:],
                                s_mean[:, 0:1], None, AluOpType.subtract)

        # pieces for d2[n,e] = |n|^2 + |e|^2 - 2 n.e  (3 accumulating matmuls)
        s_m2n = const.tile([3, NN], f32, tag="m2n")
        nc.vector.tensor_scalar_mul(s_m2n[:], s_nucT, -2.0)
        s_nsq = const.tile([3, NN], f32, tag="nsq")
        nc.vector.tensor_mul(s_nsq[:], s_nucT, s_nucT)
        s_ones3 = const.tile([3, 1], f32, tag="ones3")
        nc.vector.memset(s_ones3[:], 1.0)
        s_onesrow = const.tile([1, NN], f32, tag="onesrow")
        nc.vector.memset(s_onesrow[:], 1.0)

        ps_n2 = ps_tile([1, NN], tag="wA")
        nc.tensor.matmul(ps_n2[:], lhsT=s_ones3[:], rhs=s_nsq[:],
                         start=True, stop=True)
        s_n2 = const.tile([1, NN], f32, tag="n2")
        nc.vector.tensor_copy(s_n2[:], ps_n2[:])

        # negd[s]   : [NN, E] = -d          (DVE-path tensor_scalar scale)
        # negd_a[s] : [NN, E] = -d*ln2/1024 (ACT Exp scale against X16)
        s_negd = []
        s_negd_a = []
        for s in (0, 1):
            s_esq = const.tile([3, E], f32, tag=f"esq{s}")
            nc.vector.tensor_mul(s_esq[:], s_eT[s], s_eT[s])
            ps_e2 = ps_tile([1, E], tag="wA")
            nc.tensor.matmul(ps_e2[:], lhsT=s_ones3[:], rhs=s_esq[:],
                             start=True, stop=True)
            s_e2 = const.tile([1, E], f32, tag=f"e2{s}")
            nc.vector.tensor_copy(s_e2[:], ps_e2[:])

            ps_d2 = ps_tile([NN, E], tag="wA")
            nc.tensor.matmul(ps_d2[:], lhsT=s_m2n[:], rhs=s_eT[s],
                             start=True, stop=False)
            nc.tensor.matmul(ps_d2[:], lhsT=s_n2[:], rhs=s_onesrow[:, 0:E],
                             start=False, stop=False)
            nc.tensor.matmul(ps_d2[:], lhsT=s_onesrow[:], rhs=s_e2[:],
                             start=False, stop=True)
            nd = const.tile([NN, E], f32, tag=f"negd{s}")
            # d = exp(0.5*ln(d2)): stays inside the natural_log_exp table
            # set (sqrt would force a second ACT table load + switch).
            # Guard: the expansion |n|^2+|e|^2-2n.e can round negative for
            # near-coincident points; clamp before Ln.
            s_d2c = const.tile([NN, E], f32, tag=f"d2c{s}")
            nc.vector.tensor_scalar_max(s_d2c[:], ps_d2[:], 1e-24)
            nc.scalar.activation(nd[:], s_d2c[:], AF.Ln)
            nc.scalar.activation(nd[:], nd[:], AF.Exp, scale=0.5)
            nc.vector.tensor_scalar_mul(nd[:], nd[:], -1.0)
            nda = const.tile([NN, E], f32, tag=f"negda{s}")
            nc.vector.tensor_scalar_mul(nda[:], nd[:], LN2_1024)
            s_negd.append(nd)
            s_negd_a.append(nda)

        # fp16 feats tile [4, NN] = [charge; centered coords] for K=4 matmuls.
        # Rows 1..3 are placed by DMA (engines cannot write partition base 1).
        s_chg16 = const.tile([1, NN], f16, tag="chg16")
        nc.vector.tensor_copy(s_chg16[:], s_chg)
        s_cnuc16 = const.tile([3, NN], f16, tag="cnuc16")
        nc.vector.tensor_copy(s_cnuc16[:], s_cnuc[:])
        s_f16 = const.tile([4, NN], f16, tag="feats16")
        nc.sync.dma_start(s_f16[0:1, :], s_chg16[:])
        nc.sync.dma_start(s_f16[1:4, :], s_cnuc16[:])
        # remaining W quarters after the feats assembly DMAs
        for q in range(1, 4):
            qs = slice(q * NORB, (q + 1) * NORB)
            nc.sync.dma_start(s_w4[:, qs], d_w4[:, qs])

        # One-hot selector per (spin, electron): lhsT slice (s,e) is [128, E]
        # with column e = +1 (ACT path) or -1 (DVE path, negated encoding).
        # memsets on gpsimd (free engine).
        s_oh = const.tile([128, 2 * E * E], f16, tag="onehot")
        nc.gpsimd.memset(s_oh[:], 0.0)
        for s in (0, 1):
            for e in range(E):
                col = (s * E + e) * E + e
                val = -1.0 if e in DVE_E[s] else 1.0
                nc.gpsimd.memset(s_oh[:, col:col + 1], val)

        def oh(s, e):
            base = (s * E + e) * E
            return s_oh[:, base:base + E]

        # ---------------- zeta / pi -> X16 / piT ----------------
        # X16[s] = fp16(C_FOLD*|z|)   piT[s] = fp16(pi)
        # PSUM: two 4-bank [128, 2048] tiles (tags wA / wB); each W matrix
        # needs two of them (NORB=4096).  Sequence per bank-group is
        # serialized by the evacuation reads.
        s_X16 = []
        s_piT = []
        for s in (0, 1):
            s_X16.append(const.tile([128, NORB], f16, tag=f"x16_{s}",
                                    name=f"x16_{s}"))
            s_piT.append(const.tile([128, NORB], f16, tag=f"pit{s}",
                                    name=f"pit{s}"))

        HW = 2048  # evac half-width (4 PSUM banks)

        def w_group(s, m, half):
            """Matmuls for W matrix (s, m) covering orbital cols
            [half*2048, (half+1)*2048) into a fresh 4-bank PSUM tile."""
            tag = "wA" if (2 * s + m + half) % 2 == 0 else "wB"
            ps = ps_tile([128, HW], tag=tag)
            w_off = (2 * s + m) * NORB + half * HW
            for q in range(4):
                sl = slice(w_off + q * 512, w_off + (q + 1) * 512)
                nc.tensor.matmul(ps[:, q * 512:(q + 1) * 512],
                                 lhsT=s_f16[:], rhs=s_w4[:, sl],
                                 start=True, stop=True)
            return ps

        def evac_x16(s, half, ps, eng):
            # abs_max is not a valid tensor_scalar ALU op (walrus ISA check),
            # so all X16 evacs go through ScE AF.Abs regardless of `eng`.
            dst = s_X16[s][:, half * HW:(half + 1) * HW]
            nc.scalar.activation(dst, ps[:], AF.Abs)

        def evac_pit(s, half, ps, eng):
            dst = s_piT[s][:, half * HW:(half + 1) * HW]
            if eng == "v":
                nc.vector.tensor_copy(dst, ps[:])
            else:
                nc.scalar.copy(dst, ps[:])

        # ---------------- electron emission helpers ----------------
        # orb accumulators: two 4-bank [E, 2048] tiles per spin (tags wA/wB,
        # recycled from the W tiles once those are evacuated).
        NCHUNK = NORB // 512

        def emit_exp_act(s, e, t_dst, halves):
            if halves:
                nc.scalar.activation(t_dst[:, 0:HW], s_X16[s][:, 0:HW],
                                     AF.Exp, scale=s_negd_a[s][:, e:e + 1])
                nc.scalar.activation(t_dst[:, HW:], s_X16[s][:, HW:],
                                     AF.Exp, scale=s_negd_a[s][:, e:e + 1])
            else:
                nc.scalar.activation(t_dst[:], s_X16[s][:], AF.Exp,
                                     scale=s_negd_a[s][:, e:e + 1])

        def emit_fastexp_dve(s, e, t_i):
            nc.vector.tensor_scalar(t_i[:], s_X16[s][:],
                                    s_negd[s][:, e:e + 1], SCHRAUD_S2,
                                    AluOpType.mult, AluOpType.add)

        def emit_pimul(s, t_f):
            # in-place full-width fp16 multiply by piT (2x_1P); the deep
            # T-tile backlog keeps PE fed without needing half-granularity
            nc.vector.tensor_mul(t_f[:], t_f[:], s_piT[s][:])

        def emit_mms(s, e, t_f, ps_orb):
            first = e == 0
            last = e == E - 1
            for c in range(NCHUNK):
                dst = ps_orb[c // 4][:, (c % 4) * 512:(c % 4 + 1) * 512]
                nc.tensor.matmul(dst, lhsT=oh(s, e),
                                 rhs=t_f[:, c * 512:(c + 1) * 512],
                                 start=first, stop=last)

        def emit_electron(s, e, ps_orb, halves=False):
            if e in DVE_E[s]:
                t_i = tpool.tile([128, NORB], i16, tag="T")
                emit_fastexp_dve(s, e, t_i)
                t_f = t_i[:].bitcast(f16)
                emit_pimul(s, t_f)
                emit_mms(s, e, t_f, ps_orb)
            else:
                t_e = tpool.tile([128, NORB], f16, tag="T")
                emit_exp_act(s, e, t_e, halves)
                emit_pimul(s, t_e[:])
                emit_mms(s, e, t_e[:], ps_orb)

        # --- setup emission, interleaved with spin0's first electrons so no
        # engine starves.  X16 spin0 split DVE/ACT for head latency; the rest
        # mostly on ScE (off the hot DVE path).
        ps = w_group(0, 0, 0)
        evac_x16(0, 0, ps, "v")
        t_e0 = tpool.tile([128, NORB], f16, tag="T")
        nc.scalar.activation(t_e0[:, 0:HW], s_X16[0][:, 0:HW],
                             AF.Exp, scale=s_negd_a[0][:, 0:1])
        ps = w_group(0, 0, 1)
        evac_x16(0, 1, ps, "a")
        nc.scalar.activation(t_e0[:, HW:], s_X16[0][:, HW:],
                             AF.Exp, scale=s_negd_a[0][:, 0:1])
        ps = w_group(0, 1, 0)
        evac_pit(0, 0, ps, "v")
        ps = w_group(0, 1, 1)
        evac_pit(0, 1, ps, "a")
        emit_pimul(0, t_e0[:])

        # spin1 setup (X16 on DVE early, piT on ACT), interleaved with the
        # first spin0 electrons
        ps_orb0 = [psum.tile([E, HW], f32, tag="wA", name="orb0A"),
                   psum.tile([E, HW], f32, tag="wB", name="orb0B")]
        emit_mms(0, 0, t_e0[:], ps_orb0)
        emit_electron(0, 1, ps_orb0)
        ps = w_group(1, 0, 0)
        evac_x16(1, 0, ps, "v")
        emit_electron(0, 2, ps_orb0)
        ps = w_group(1, 0, 1)
        evac_x16(1, 1, ps, "v")
        emit_electron(0, 3, ps_orb0)
        ps = w_group(1, 1, 0)
        evac_pit(1, 0, ps, "a")
        emit_electron(0, 4, ps_orb0)
        ps = w_group(1, 1, 1)
        evac_pit(1, 1, ps, "a")
        for e in range(5, E):
            emit_electron(0, e, ps_orb0)

        # spin0 orb evacuation ([E, 2048] x2, both on ScE -- DVE is the
        # hotter engine mid-kernel) + DMA out
        s_o0 = [opool.tile([E, HW], f16, tag="osb", name=f"o0_{i}")
                for i in range(2)]
        nc.scalar.copy(s_o0[0][:], ps_orb0[0][:])
        nc.scalar.copy(s_o0[1][:], ps_orb0[1][:])
        nc.sync.dma_start(d_out[0][:, 0:HW], s_o0[0][:])
        nc.sync.dma_start(d_out[0][:, HW:], s_o0[1][:])

        # ---------------- spin 1 main loop ----------------
        ps_orb1 = [psum.tile([E, HW], f32, tag="wA", name="orb1A"),
                   psum.tile([E, HW], f32, tag="wB", name="orb1B")]
        for e in range(E):
            emit_electron(1, e, ps_orb1, halves=(e in (0, E - 1)))

        # tail evac split across both engines for latency (ACT is idle by
        # now); both DMAs on the sync HWDGE queue (the gpsimd SWDGE queue
        # measured ~2x slower per packet)
        s_o1 = [opool.tile([E, HW], f16, tag="osb", name=f"o1_{i}")
                for i in range(2)]
        nc.vector.tensor_copy(s_o1[0][:], ps_orb1[0][:])
        nc.scalar.copy(s_o1[1][:], ps_orb1[1][:])
        nc.sync.dma_start(d_out[1][:, 0:HW], s_o1[0][:])
        nc.sync.dma_start(d_out[1][:, HW:], s_o1[1][:])

    return nc


def _get_module():
    if "nc" not in _CACHE:
        _CACHE["nc"] = _build_module()
    return _CACHE["nc"]


def kernel(**inputs) -> np.ndarray:
    global LAST_RESULTS
    nc = _get_module()
    from concourse.bass_utils import run_bass_kernel_spmd

    up = np.ascontiguousarray(np.asarray(inputs["up_coords"], dtype=np.float32))
    down = np.ascontiguousarray(np.asarray(inputs["down_coords"], dtype=np.float32))
    nuc = np.asarray(inputs["nuc_coords"], dtype=np.float32)
    chg = np.asarray(inputs["nuc_charges"], dtype=np.float32)
    w = {
        k: np.ascontiguousarray(np.asarray(inputs[k], dtype=np.float32))
        for k in ("W_pi_up", "W_zeta_up", "W_pi_down", "W_zeta_down")
    }
    nucT = nuc.T                                  # [3, 128]

    # zeta sources (W_pi_*) host-prescaled by C_FOLD for the fp16 X16 layout
    wmats = [w["W_pi_up"] * np.float32(C_FOLD), w["W_zeta_up"],
             w["W_pi_down"] * np.float32(C_FOLD), w["W_zeta_down"]]
    wsplit = {
        "w4": np.ascontiguousarray(
            np.concatenate(wmats, axis=1).astype(np.float16)),
    }

    in_maps = []
    for c in range(N_CORES):
        sl = slice(c * E_PER_CORE, (c + 1) * E_PER_CORE)
        small = np.zeros((3, 2 * NN + 2 * E_PER_CORE), dtype=np.float32)
        small[:, 0:NN] = nucT
        small[0, NN:2 * NN] = chg
        small[:, 2 * NN:2 * NN + E_PER_CORE] = up[sl].T
        small[:, 2 * NN + E_PER_CORE:] = down[sl].T
        in_maps.append({"small": small, **wsplit})

    res = run_bass_kernel_spmd(nc, in_maps, core_ids=list(range(N_CORES)))
    LAST_RESULTS = res

    # gather: per-core slab is already [2, e_local, orbital]
    orb = np.empty((2, NE, NORB), dtype=np.float32)
    for c in range(N_CORES):
        a = np.asarray(res.results[c]["out"])            # [2, E, NORB] f16
        orb[:, c * E_PER_CORE:(c + 1) * E_PER_CORE, :] = a.astype(np.float32)

    # [2, n_e, n_det*max_e] -> [2, n_det, n_e, max_e]
    out = orb.reshape(2, NE, NDET, NE).swapaxes(1, 2)
    return np.ascontiguousarray(out)
